# revision 44
# baseline (speedup 1.0000x reference)
"""Trainium2 Bass kernel for EpiLinear (epinet + prior-ensemble MLP).

Strategy (data-parallel over batch, params replicated; per core BL=256 rows):
  - Epinet L1 split: A = xf @ Wep1[:2048] computed once per batch row
    (8x fewer FLOPs than the naive [B*n, 2080] GEMM); bep1 folded into the
    A GEMM via a K=1 ones-row matmul; Bz = z @ Wep1[2048:] runs in fp8
    DoubleRow mode (z and the 32 z-weights quantize harmlessly: Bz is
    ~12% of h's magnitude).
  - h = relu(A + Bz) via Pool/DVE tensor ops reading both PSUM banks
    directly; the last hid-tile's A GEMM is split into b-halves so its
    elementwise work overlaps the PE.
  - Epinet L2 streams W2 against stationary h-slices: out2^T[r, o] in
    N=32-column matmuls; the prior ensemble output p (32 tiny MLPs
    1024->5->5->1 as block-diagonal GEMMs) is transposed on the PE and
    accumulated into the same PSUM tile via an identity matmul.
  - Final contraction over the 32 noise dims: tensor-tensor multiply with
    z ([r, o] layout) on Pool and a grouped tensor_reduce on DVE,
    pipelined per 512-row chunk under the PE tail.
  - The PE is kept continuously busy from ~700ns (warm-up matmuls on
    memset data hold the p-state ramp) and bulk DMAs are spread across
    the SP/Act/Pool queues in PE consumption order.
"""

import time

import numpy as np
import ml_dtypes

import concourse.bacc as bacc
import concourse.mybir as mybir
import concourse.tile as tile
from concourse.bass_utils import run_bass_kernel_spmd

F32 = mybir.dt.float32
BF16 = mybir.dt.bfloat16
FP8 = mybir.dt.float8e4
RELU = mybir.ActivationFunctionType.Relu
COPY = mybir.ActivationFunctionType.Copy
IDENT = mybir.ActivationFunctionType.Identity
ADD = mybir.AluOpType.add
MULT = mybir.AluOpType.mult
DR = mybir.MatmulPerfMode.DoubleRow

NPDT = ml_dtypes.bfloat16
NP8 = ml_dtypes.float8_e4m3

N_CORES = 8
B, N_Z, ND, SD, HD = 2048, 8, 32, 1024, 1024
EH = 512                  # epinet hidden
XF = SD + HD              # 2048 concat(x, feature) features
BL = B // N_CORES         # 256 batch rows per core
R = BL * N_Z              # 2048 epinet rows per core (r = n*BL + b, n-major)
PHF = 160                 # 32 ensembles * 5 prior hidden, flattened
KT = XF // 128            # 16 k-tiles over xf features
MT = EH // 128            # 4 hid tiles of epinet hidden
RC = R // 512             # 4 chunks of 512 epinet rows
RT = R // 128             # 16 row-tiles for the L2/final stage

# small bf16 param block column offsets: bep1 (row 0) | w2 | Id | wp2a |
# wp2b | wp3a | wp3b  (wp1 rides in its own tensor, z+w1z in fp8 z8w)
PK_B1, PK_W2, PK_ID = 0, 512, 640
PK_W2A, PK_W2B, PK_W3A, PK_W3B, PK_COLS = 768, 928, 1088, 1120, 1152

# zrb fp32 bias columns (appended to the [128, 512] z block)
ZB_BP1A, ZB_BP1B, ZB_BP2A, ZB_BP2B, ZB_P, ZB_COLS = 512, 513, 514, 515, 516, 520

N_WARM = 2                # PE warm-up matmuls (hold the p-state ramp clock)

DEBUG_TAPS = False        # extra DRAM outputs for numerical bisection

_CACHE = {}


def _build():
    nc = bacc.Bacc("TRN2", target_bir_lowering=False, debug=False,
                   num_devices=N_CORES)
    f = lambda name, shape, dt: nc.dram_tensor(name, shape, dt, kind="ExternalInput").ap()
    xfT = f("xfT", [128, KT * BL], BF16)   # xf.T, SBUF-layout swizzled
    w1 = f("w1", [128, MT, KT * 128], BF16)  # Wep1[:2048] SBUF-layout swizzled
    z8w = f("z8w", [128, 2048], FP8)       # fp8 z-pairs + w1z-pairs per rc
    zrb = f("zrb", [128, ZB_COLS], F32)    # z in [r, o] layout + fp32 biases
    pk = f("pk", [128, PK_COLS], BF16)     # small bf16 params
    wp1 = f("wp1", [128, 8 * PHF], BF16)   # prior L1 weights, swizzled
    out = nc.dram_tensor("out", [128, RT], F32, kind="ExternalOutput").ap()

    with tile.TileContext(nc) as tc:
        with (
            tc.tile_pool(name="const", bufs=1) as cp,
            tc.tile_pool(name="work", bufs=1) as wk,
            tc.tile_pool(name="tmp", bufs=4) as tp,
            tc.tile_pool(name="ps_a", bufs=1, space="PSUM") as ps_a,
            tc.tile_pool(name="ps_bz", bufs=5, space="PSUM") as ps_bz,
            tc.tile_pool(name="ps_sm", bufs=1, space="PSUM") as ps_sm,
            tc.tile_pool(name="ps_p", bufs=1, space="PSUM") as ps_p,
        ):
            # ---- SBUF tiles -------------------------------------------------
            xfT_sb = cp.tile([128, KT * BL], BF16)     # [p, (k b)]
            w1_sb = cp.tile([128, MT * KT * 128], BF16)  # [p, (m k h)]
            z8w_sb = cp.tile([128, 2048], FP8)
            zrb_sb = cp.tile([128, ZB_COLS], F32)
            pk_sb = cp.tile([128, PK_COLS], BF16)
            wp1_sb = cp.tile([128, 8 * PHF], BF16)
            ones_sb = cp.tile([1, BL], BF16)
            jw_sb = cp.tile([1, 512], BF16)

            h_sb = [wk.tile([128, R], BF16, name=f"h{m}") for m in range(MT)]
            A_sb = wk.tile([128, MT * BL], BF16)       # A + bep1, bf16 copy
            h1a_sb = wk.tile([128, BL], BF16)
            h1b_sb = wk.tile([32, BL], BF16)
            h2a_sb = wk.tile([128, BL], BF16)
            h2b_sb = wk.tile([32, BL], BF16)
            pS_sb = wk.tile([32, BL], BF16)            # prior out [e, b]
            pT_sb = wk.tile([128, 2 * ND], BF16)       # prior out [b, e] 2 halves
            gm_sb = wk.tile([128, RT * ND], BF16)
            res_sb = wk.tile([128, RT], F32)

            x3 = xfT_sb[:].rearrange("p (k b) -> p k b", b=BL)
            w13 = w1_sb[:].rearrange("p (m k h) -> p m k h", m=MT, h=128)
            b1_v = pk_sb[0:1, PK_B1:PK_B1 + EH]
            w23 = pk_sb[:, PK_W2:PK_ID].rearrange("p (k o) -> p k o", o=ND)
            id_v = pk_sb[:, PK_ID:PK_W2A]
            wp2a_v = pk_sb[:, PK_W2A:PK_W2B]
            wp2b_v = pk_sb[0:32, PK_W2B:PK_W3A]
            wp3a_v = pk_sb[:, PK_W3A:PK_W3B]
            wp3b_v = pk_sb[0:32, PK_W3B:PK_COLS]
            wp13 = wp1_sb[:].rearrange("p (k g) -> p k g", g=PHF)

            # warm-up sources (memset, no DMA dependency); jw on Pool so it
            # lands before the Pool DMA queue starts
            nc.gpsimd.memset(jw_sb[:], 0.0)
            nc.vector.memset(ones_sb[:], 1.0)

            # ---- DMAs spread over 3 queues, ordered by PE consumption ------
            # SP:   z8w -> w1[m0] -> w1[m2] -> w1[m3] -> out
            # Pool: bep1 -> xf c0 -> xf c1 -> w1[m1]  (Pool h-adds from ~6us)
            # Act:  [LoadActFuncSet] -> wp1 -> xf c2 -> xf c3 -> zrb -> pk2
            def _xf(q, c):
                q.dma_start(xfT_sb[:, 4 * c * BL:(4 * c + 4) * BL],
                            xfT[:, 4 * c * BL:(4 * c + 4) * BL])
            def _w1(q, c):
                q.dma_start(w1_sb[:, c * KT * 128:(c + 1) * KT * 128],
                            w1[:, c, :])
            nc.sync.dma_start(z8w_sb[:], z8w[:])
            nc.gpsimd.dma_start(pk_sb[0:1, PK_B1:PK_W2], pk[0:1, PK_B1:PK_W2])
            _xf(nc.gpsimd, 0)
            _w1(nc.sync, 0)
            nc.scalar.dma_start(wp1_sb[:], wp1[:])
            _xf(nc.gpsimd, 1)
            _w1(nc.gpsimd, 1)
            _xf(nc.scalar, 2)
            _xf(nc.scalar, 3)
            _w1(nc.sync, 2)
            _w1(nc.sync, 3)
            nc.scalar.dma_start(zrb_sb[:], zrb[:])
            nc.scalar.dma_start(pk_sb[:, PK_W2:], pk[:, PK_W2:])

            # ---- PE warm-up: junk matmuls on memset data -------------------
            psj = ps_sm.tile([1, 512], F32, tag="pp", name="junk")
            for i in range(N_WARM):
                nc.tensor.matmul(psj[0:1, :], jw_sb[0:1, 0:1], jw_sb[0:1, :],
                                 start=True, stop=True)

            # ---- epinet L1 + prior -----------------------------------------
            psz = {}
            psA = {}

            def bz(m):
                for rc in range(RC):
                    t = ps_bz.tile([128, 512], F32, tag="pz",
                                   name=f"pz{m}_{rc}")
                    psz[(m, rc)] = t
                    w8v = z8w_sb[32 * rc:32 * rc + 16, 1024:2048].rearrange(
                        "p (i h) -> p i h", i=2)[:, :, 128 * m:128 * m + 128]
                    z8v = z8w_sb[32 * rc:32 * rc + 16, 0:1024].rearrange(
                        "p (i j) -> p i j", i=2)
                    nc.tensor.matmul(t[:, :], w8v, z8v, start=True, stop=True,
                                     perf_mode=DR, tile_position=(32 * rc, 0))

            def a_gemm(m, ks, stop=False, bias=False):
                if bias:
                    ps = ps_a.tile([128, BL], F32, tag="pa", name=f"pA{m}")
                    psA[m] = ps
                    nc.tensor.matmul(psA[m][:, :],
                                     b1_v[:, 128 * m:128 * m + 128],
                                     ones_sb[0:1, :], start=True, stop=False)
                for i, k in enumerate(ks):
                    nc.tensor.matmul(
                        psA[m][:, :], w13[:, m, k, :], x3[:, k, :],
                        start=False, stop=(stop and i == len(ks) - 1))

            def h_elem(m, act_rcs=()):
                nc.scalar.activation(A_sb[:, BL * m:BL * (m + 1)],
                                     psA[m][:, :], COPY)
                Ab = A_sb[:, BL * m:BL * (m + 1)].unsqueeze(1).broadcast_to(
                    (128, 2, BL))
                us = {}
                for rc in act_rcs:
                    u = tp.tile([128, 512], BF16, tag="u", name=f"u{m}_{rc}")
                    us[rc] = u
                    nc.scalar.activation(u[:], psz[(m, rc)][:, :], COPY)
                ts = []
                for rc in range(RC):
                    t = tp.tile([128, 512], BF16, tag="t")
                    ts.append(t)
                    eng = nc.gpsimd if rc in us else nc.vector
                    srcz = us[rc][:] if rc in us else psz[(m, rc)][:, :]
                    eng.tensor_tensor(
                        t[:].rearrange("p (a b) -> p a b", a=2),
                        srcz.rearrange("p (a b) -> p a b", a=2),
                        Ab, op=ADD)
                for rc in range(RC):
                    eng = nc.vector if rc in us else nc.gpsimd
                    if rc in us:
                        nc.vector.tensor_scalar_max(
                            h_sb[m][:, 512 * rc:512 * rc + 512], ts[rc][:], 0.0)
                    else:
                        nc.gpsimd.tensor_scalar_max(
                            h_sb[m][:, 512 * rc:512 * rc + 512], ts[rc][:], 0.0)

            # m0 first: Bz only needs z8w; A(m0) follows w1[m0]
            bz(0)
            a_gemm(0, [0, 1, 2, 3, 4, 5, 6, 7], bias=True)
            a_gemm(0, [8, 9, 10, 11, 12, 13, 14, 15], stop=True)
            h_elem(0)

            # prior h1 (c0/c1 + wp1 are on-chip by now)
            ps1 = ps_sm.tile([128, 2 * BL], F32, tag="pp", name="pp1")
            for g, (mp, m0) in enumerate([(128, 0), (32, 128)]):
                o = ps1[0:mp, BL * g:BL * g + BL]
                for k in range(8):
                    nc.tensor.matmul(
                        o, wp13[:, k, m0:m0 + mp], x3[:, k, :],
                        start=(k == 0), stop=(k == 7))

            nc.scalar.activation(h1a_sb[:], ps1[0:128, 0:BL], RELU,
                                 bias=zrb_sb[:, ZB_BP1A:ZB_BP1A + 1])
            nc.scalar.activation(h1b_sb[:], ps1[0:32, BL:2 * BL], RELU,
                                 bias=zrb_sb[0:32, ZB_BP1B:ZB_BP1B + 1])

            # m1: Bz between the A halves so m0's psz slots drain first
            a_gemm(1, [0, 1, 2, 3, 4, 5, 6, 7], bias=True)
            bz(1)
            a_gemm(1, [8, 9, 10, 11, 12, 13, 14, 15], stop=True)
            h_elem(1)

            # h2 = relu(h1 @ wp2 + bp2)   (block-diag dense)
            ps2 = ps_sm.tile([128, 2 * BL], F32, tag="pp", name="pp2")
            for g, (mp, m0) in enumerate([(128, 0), (32, 128)]):
                o = ps2[0:mp, BL * g:BL * g + BL]
                nc.tensor.matmul(o, wp2a_v[:, m0:m0 + mp],
                                 h1a_sb[:], start=True, stop=False)
                nc.tensor.matmul(o, wp2b_v[:, m0:m0 + mp],
                                 h1b_sb[:], start=False, stop=True)
            nc.scalar.activation(h2a_sb[:], ps2[0:128, 0:BL], RELU,
                                 bias=zrb_sb[:, ZB_BP2A:ZB_BP2A + 1])
            nc.scalar.activation(h2b_sb[:], ps2[0:32, BL:2 * BL], RELU,
                                 bias=zrb_sb[0:32, ZB_BP2B:ZB_BP2B + 1])

            # m2: rc2/rc3 h-adds go via Act-copy + Pool so DVE keeps up
            a_gemm(2, [0, 1, 2, 3, 4, 5, 6, 7], bias=True)
            bz(2)
            a_gemm(2, [8, 9, 10, 11, 12, 13, 14, 15], stop=True)
            h_elem(2, act_rcs=(3,))

            bz(3)

            # out2 k0/k1 passes (need only h(m0)/h(m1)).
            # NOTE: start=True zeroes the whole 2KB PSUM region lazily, so
            # only the very first call starts; later chains' first touch
            # inherits the pending-zero (overwrite) semantics.
            pso = ps_sm.tile([128, RT * ND], F32, tag="pp", name="po")

            def o2pass(k):
                for t in range(RT):
                    nc.tensor.matmul(
                        pso[:, ND * t:ND * t + ND],
                        h_sb[k][:, 128 * t:128 * t + 128], w23[:, k, :],
                        start=(k == 0 and t == 0), stop=False,
                        skip_group_check=True)

            # m3: single A chain; q1-q3 psz pre-copied by Act during the
            # chain, then Pool adds; q0 added by DVE straight from PSUM.
            a_gemm(3, [0, 1, 2, 3, 4, 5, 6, 7], bias=True)
            u3 = {}
            for rc in (1, 2):
                u = tp.tile([128, 512], BF16, tag="u", name=f"u3_{rc}")
                u3[rc] = u
                nc.scalar.activation(u[:], psz[(3, rc)][:, :], COPY)
            a_gemm(3, [8, 9, 10, 11, 12, 13, 14, 15], stop=True)

            A3c = A_sb[:, 3 * BL:4 * BL]
            nc.scalar.activation(A3c, psA[3][:, :], COPY)
            u = tp.tile([128, 512], BF16, tag="u", name="u3_3")
            u3[3] = u
            nc.scalar.activation(u[:], psz[(3, 3)][:, :], COPY)
            Ab3 = A3c.unsqueeze(1).broadcast_to((128, 2, BL))
            t3s = {}
            for rc in range(RC):
                t3 = tp.tile([128, 512], BF16, tag="t", name=f"t3_{rc}")
                t3s[rc] = t3
                eng = nc.vector if rc < 1 else nc.gpsimd
                srcz = (psz[(3, rc)][:, :] if rc < 1 else u3[rc][:])
                eng.tensor_tensor(
                    t3[:].rearrange("p (a b) -> p a b", a=2),
                    srcz.rearrange("p (a b) -> p a b", a=2), Ab3, op=ADD)
            relu_eng = [nc.vector, nc.vector, nc.scalar, nc.gpsimd]
            for rc in range(RC):
                dst = h_sb[3][:, 512 * rc:512 * rc + 512]
                if relu_eng[rc] is nc.scalar:
                    nc.scalar.activation(dst, t3s[rc][:], RELU)
                else:
                    relu_eng[rc].tensor_scalar_max(dst, t3s[rc][:], 0.0)

            # prior tail: p + transpose overlap the m3 elementwise above
            psp = ps_p.tile([128, BL], F32, tag="px", name="ppp")
            nc.tensor.matmul(psp[0:32, 0:BL], wp3a_v[:], h2a_sb[:],
                             start=True, stop=False)
            nc.tensor.matmul(psp[0:32, 0:BL], wp3b_v[:], h2b_sb[:],
                             start=False, stop=True)
            nc.scalar.activation(pS_sb[:], psp[0:32, 0:BL], IDENT,
                                 bias=zrb_sb[0:32, ZB_P:ZB_P + 1])
            psT = ps_p.tile([128, 2 * ND], BF16, tag="px", name="psT")
            for half in range(2):
                nc.tensor.transpose(
                    psT[:, ND * half:ND * half + ND],
                    pS_sb[:, 128 * half:128 * half + 128],
                    id_v[0:32, 0:32])
            nc.scalar.activation(pT_sb[:], psT[:, :], COPY)

            o2pass(0)
            o2pass(1)
            o2pass(2)

            # ---- per-chunk tail: k3+Id, then Pool gm-mult, DVE row-sum ------
            for q in range(4):
                for t in range(4 * q, 4 * q + 4):
                    o = pso[:, ND * t:ND * t + ND]
                    nc.tensor.matmul(
                        o, h_sb[3][:, 128 * t:128 * t + 128],
                        w23[:, 3, :], start=False, stop=False,
                        skip_group_check=True)
                    nc.tensor.matmul(
                        o, id_v[:], pT_sb[:, ND * (t % 2):ND * (t % 2) + ND],
                        start=False, stop=True, skip_group_check=True)
                sl = slice(4 * ND * q, 4 * ND * (q + 1))
                nc.vector.tensor_tensor(gm_sb[:, sl], pso[:, sl],
                                        zrb_sb[:, sl], op=MULT)
                nc.vector.tensor_reduce(
                    res_sb[:, 4 * q:4 * q + 4],
                    gm_sb[:, sl].rearrange("p (t o) -> p t o", o=ND),
                    axis=mybir.AxisListType.X, op=ADD)
            nc.sync.dma_start(out[:], res_sb[:])

            if DEBUG_TAPS:
                d_h0 = nc.dram_tensor("d_h0", [128, R], BF16,
                                      kind="ExternalOutput").ap()
                d_pS = nc.dram_tensor("d_pS", [32, BL], BF16,
                                      kind="ExternalOutput").ap()
                d_gm = nc.dram_tensor("d_gm", [128, RT * ND], BF16,
                                      kind="ExternalOutput").ap()
                nc.sync.dma_start(d_h0[:], h_sb[0][:])
                nc.sync.dma_start(d_pS[:], pS_sb[:])
                nc.sync.dma_start(d_gm[:], gm_sb[:])

    nc.compile()
    return nc


def _prep(x, feature, z, Wep1, bep1, Wep2, bep2, Wp1, bp1, Wp2, bp2, Wp3, bp3):
    """Host-side weight/layout prep shared across cores."""
    xfT = np.ascontiguousarray(
        np.concatenate([x, feature], axis=1).T.astype(NPDT))  # [XF, B]
    # swizzle w1 into SBUF layout [p, m, (k h)]
    w1 = np.ascontiguousarray(
        np.asarray(Wep1, np.float32)[:XF].astype(NPDT)
        .reshape(KT, 128, MT, 128).transpose(1, 2, 0, 3)
        .reshape(128, MT, KT * 128))

    pk = np.zeros((128, PK_COLS), NPDT)
    pk[0, PK_B1:PK_W2] = np.asarray(bep1, np.float32)
    pk[:, PK_W2:PK_ID] = (np.asarray(Wep2, np.float32)
                          .reshape(4, 128, ND).transpose(1, 0, 2)
                          .reshape(128, 4 * ND))
    pk[:, PK_ID:PK_W2A] = np.eye(128, dtype=np.float32)
    wp2 = np.zeros((PHF, PHF), np.float32)
    wp3 = np.zeros((PHF, ND), np.float32)
    for e in range(ND):
        wp2[5 * e:5 * e + 5, 5 * e:5 * e + 5] = Wp2[e]
        wp3[5 * e:5 * e + 5, e] = np.asarray(Wp3)[e, :, 0]
    pk[:, PK_W2A:PK_W2B] = wp2[0:128]
    pk[0:32, PK_W2B:PK_W3A] = wp2[128:160]
    pk[:, PK_W3A:PK_W3B] = wp3[0:128]
    pk[0:32, PK_W3B:PK_COLS] = wp3[128:160]

    wp1 = np.ascontiguousarray(
        np.asarray(Wp1, np.float32).transpose(1, 0, 2).reshape(SD, PHF)
        .reshape(8, 128, PHF).transpose(1, 0, 2)
        .reshape(128, 8 * PHF).astype(NPDT))

    w1z8 = (np.asarray(Wep1, np.float32)[XF:]
            .reshape(16, 2, EH).reshape(16, 2 * EH).astype(NP8))

    bp1f = np.asarray(bp1, np.float32).reshape(PHF)
    bp2f = np.asarray(bp2, np.float32).reshape(PHF)
    pbias = np.asarray(bep2, np.float32) + np.asarray(bp3, np.float32)[:, 0]

    shared = dict(w1=w1, pk=pk, wp1=wp1)
    in_maps = []
    for c in range(N_CORES):
        sl = slice(c * BL, (c + 1) * BL)
        zTf = np.asarray(z)[sl].transpose(1, 0, 2).reshape(R, ND).T  # [32, R]
        z8w = np.zeros((128, 2048), NP8)
        for rc in range(RC):
            z8w[32 * rc:32 * rc + 16, 0:1024] = (
                zTf[:, 512 * rc:512 * rc + 512]
                .reshape(16, 2, 512).reshape(16, 1024).astype(NP8))
            z8w[32 * rc:32 * rc + 16, 1024:2048] = w1z8
        zrb = np.zeros((128, ZB_COLS), np.float32)
        # zrb[p, (t, o)] = z[o, 128 t + p]
        zrb[:, 0:RT * ND] = (zTf.T.reshape(RT, 128, ND)
                             .transpose(1, 0, 2).reshape(128, RT * ND))
        zrb[:, ZB_BP1A] = bp1f[:128]
        zrb[0:32, ZB_BP1B] = bp1f[128:]
        zrb[:, ZB_BP2A] = bp2f[:128]
        zrb[0:32, ZB_BP2B] = bp2f[128:]
        zrb[0:32, ZB_P] = pbias
        m = dict(shared)
        m["xfT"] = np.ascontiguousarray(
            xfT[:, sl].reshape(KT, 128, BL).transpose(1, 0, 2)
            .reshape(128, KT * BL))
        m["z8w"] = z8w
        m["zrb"] = zrb
        in_maps.append(m)
    return in_maps


def kernel(**inputs):
    if "nc" not in _CACHE:
        _CACHE["nc"] = _build()
    nc = _CACHE["nc"]
    in_maps = _prep(**inputs)
    last_err = None
    for _attempt in range(3):
        try:
            res = run_bass_kernel_spmd(nc, in_maps, list(range(N_CORES)))
            full = np.empty((B, N_Z, 1), np.float32)
            for c in range(N_CORES):
                S = np.asarray(res.results[c]["out"])  # [128, RT]
                r = S.T.reshape(R)                     # r = 128 t + p
                full[c * BL:(c + 1) * BL, :, 0] = r.reshape(N_Z, BL).T
            return full
        except Exception as e:  # transient device/transfer hiccups
            last_err = e
            time.sleep(5.0 * (_attempt + 1))
    raise last_err


# revision 46
# speedup vs baseline: 1.0116x; 1.0116x over previous
"""Trainium2 Bass kernel for EpiLinear (epinet + prior-ensemble MLP).

Strategy (data-parallel over batch, params replicated; per core BL=256 rows):
  - Epinet L1 split: A = xf @ Wep1[:2048] computed once per batch row
    (8x fewer FLOPs than the naive [B*n, 2080] GEMM); bep1 folded into the
    A GEMM via a K=1 ones-row matmul; Bz = z @ Wep1[2048:] runs in fp8
    DoubleRow mode (z and the 32 z-weights quantize harmlessly: Bz is
    ~12% of h's magnitude).
  - h = relu(A + Bz) via Pool/DVE tensor ops reading both PSUM banks
    directly; the last hid-tile's A GEMM is split into b-halves so its
    elementwise work overlaps the PE.
  - Epinet L2 streams W2 against stationary h-slices: out2^T[r, o] in
    N=32-column matmuls; the prior ensemble output p (32 tiny MLPs
    1024->5->5->1 as block-diagonal GEMMs) is transposed on the PE and
    accumulated into the same PSUM tile via an identity matmul.
  - Final contraction over the 32 noise dims: tensor-tensor multiply with
    z ([r, o] layout) on Pool and a grouped tensor_reduce on DVE,
    pipelined per 512-row chunk under the PE tail.
  - The PE is kept continuously busy from ~700ns (warm-up matmuls on
    memset data hold the p-state ramp) and bulk DMAs are spread across
    the SP/Act/Pool queues in PE consumption order.
"""

import time

import numpy as np
import ml_dtypes

import concourse.bacc as bacc
import concourse.mybir as mybir
import concourse.tile as tile
from concourse.bass_utils import run_bass_kernel_spmd

F32 = mybir.dt.float32
BF16 = mybir.dt.bfloat16
FP8 = mybir.dt.float8e4
RELU = mybir.ActivationFunctionType.Relu
COPY = mybir.ActivationFunctionType.Copy
IDENT = mybir.ActivationFunctionType.Identity
ADD = mybir.AluOpType.add
MULT = mybir.AluOpType.mult
DR = mybir.MatmulPerfMode.DoubleRow

NPDT = ml_dtypes.bfloat16
NP8 = ml_dtypes.float8_e4m3

N_CORES = 8
B, N_Z, ND, SD, HD = 2048, 8, 32, 1024, 1024
EH = 512                  # epinet hidden
XF = SD + HD              # 2048 concat(x, feature) features
BL = B // N_CORES         # 256 batch rows per core
R = BL * N_Z              # 2048 epinet rows per core (r = n*BL + b, n-major)
PHF = 160                 # 32 ensembles * 5 prior hidden, flattened
KT = XF // 128            # 16 k-tiles over xf features
MT = EH // 128            # 4 hid tiles of epinet hidden
RC = R // 512             # 4 chunks of 512 epinet rows
RT = R // 128             # 16 row-tiles for the L2/final stage

# small bf16 param block column offsets: bep1 (row 0) | w2 | Id | wp2a |
# wp2b | wp3a | wp3b  (wp1 rides in its own tensor, z+w1z in fp8 z8w)
PK_B1, PK_W2, PK_ID = 0, 512, 640
PK_W2A, PK_W2B, PK_W3A, PK_W3B, PK_COLS = 768, 928, 1088, 1120, 1152

# zrb fp32 bias columns (appended to the [128, 512] z block)
ZB_BP1A, ZB_BP1B, ZB_BP2A, ZB_BP2B, ZB_P, ZB_COLS = 512, 513, 514, 515, 516, 520

N_WARM = 2                # PE warm-up matmuls (hold the p-state ramp clock)

DEBUG_TAPS = False        # extra DRAM outputs for numerical bisection

_CACHE = {}


def _build():
    nc = bacc.Bacc("TRN2", target_bir_lowering=False, debug=False,
                   num_devices=N_CORES)
    f = lambda name, shape, dt: nc.dram_tensor(name, shape, dt, kind="ExternalInput").ap()
    xfT = f("xfT", [128, KT * BL], BF16)   # xf.T, SBUF-layout swizzled
    w1 = f("w1", [128, MT, KT * 128], BF16)  # Wep1[:2048] SBUF-layout swizzled
    z8w = f("z8w", [128, 2048], FP8)       # fp8 z-pairs + w1z-pairs per rc
    zrb = f("zrb", [128, ZB_COLS], F32)    # z in [r, o] layout + fp32 biases
    pk = f("pk", [128, PK_COLS], BF16)     # small bf16 params
    wp1 = f("wp1", [128, 8 * PHF], BF16)   # prior L1 weights, swizzled
    out = nc.dram_tensor("out", [128, RT], F32, kind="ExternalOutput").ap()

    with tile.TileContext(nc) as tc:
        with (
            tc.tile_pool(name="const", bufs=1) as cp,
            tc.tile_pool(name="work", bufs=1) as wk,
            tc.tile_pool(name="tmp", bufs=4) as tp,
            tc.tile_pool(name="ps_a", bufs=1, space="PSUM") as ps_a,
            tc.tile_pool(name="ps_bz", bufs=5, space="PSUM") as ps_bz,
            tc.tile_pool(name="ps_sm", bufs=1, space="PSUM") as ps_sm,
            tc.tile_pool(name="ps_p", bufs=1, space="PSUM") as ps_p,
        ):
            # ---- SBUF tiles -------------------------------------------------
            xfT_sb = cp.tile([128, KT * BL], BF16)     # [p, (k b)]
            w1_sb = cp.tile([128, MT * KT * 128], BF16)  # [p, (m k h)]
            z8w_sb = cp.tile([128, 2048], FP8)
            zrb_sb = cp.tile([128, ZB_COLS], F32)
            pk_sb = cp.tile([128, PK_COLS], BF16)
            wp1_sb = cp.tile([128, 8 * PHF], BF16)
            ones_sb = cp.tile([1, BL], BF16)
            jw_sb = cp.tile([1, 512], BF16)

            h_sb = [wk.tile([128, R], BF16, name=f"h{m}") for m in range(MT)]
            A_sb = wk.tile([128, MT * BL], BF16)       # A + bep1, bf16 copy
            h1a_sb = wk.tile([128, BL], BF16)
            h1b_sb = wk.tile([32, BL], BF16)
            h2a_sb = wk.tile([128, BL], BF16)
            h2b_sb = wk.tile([32, BL], BF16)
            pS_sb = wk.tile([32, BL], BF16)            # prior out [e, b]
            pT_sb = wk.tile([128, 2 * ND], BF16)       # prior out [b, e] 2 halves
            gm_sb = wk.tile([128, RT * ND], BF16)
            res_sb = wk.tile([128, RT], F32)

            x3 = xfT_sb[:].rearrange("p (k b) -> p k b", b=BL)
            w13 = w1_sb[:].rearrange("p (m k h) -> p m k h", m=MT, h=128)
            b1_v = pk_sb[0:1, PK_B1:PK_B1 + EH]
            w23 = pk_sb[:, PK_W2:PK_ID].rearrange("p (k o) -> p k o", o=ND)
            id_v = pk_sb[:, PK_ID:PK_W2A]
            wp2a_v = pk_sb[:, PK_W2A:PK_W2B]
            wp2b_v = pk_sb[0:32, PK_W2B:PK_W3A]
            wp3a_v = pk_sb[:, PK_W3A:PK_W3B]
            wp3b_v = pk_sb[0:32, PK_W3B:PK_COLS]
            wp13 = wp1_sb[:].rearrange("p (k g) -> p k g", g=PHF)

            # warm-up sources (memset, no DMA dependency); jw on Pool so it
            # lands before the Pool DMA queue starts
            nc.gpsimd.memset(jw_sb[:], 0.0)
            nc.vector.memset(ones_sb[:], 1.0)

            # ---- DMAs spread over 3 queues, ordered by PE consumption ------
            # SP:   z8w -> w1[m0] -> w1[m2] -> w1[m3] -> out
            # Pool: bep1 -> xf c0 -> xf c1 -> w1[m1]  (Pool h-adds from ~6us)
            # Act:  [LoadActFuncSet] -> wp1 -> xf c2 -> xf c3 -> zrb -> pk2
            def _xf(q, c):
                q.dma_start(xfT_sb[:, 4 * c * BL:(4 * c + 4) * BL],
                            xfT[:, 4 * c * BL:(4 * c + 4) * BL])
            def _w1(q, c):
                q.dma_start(w1_sb[:, c * KT * 128:(c + 1) * KT * 128],
                            w1[:, c, :])
            nc.sync.dma_start(z8w_sb[:], z8w[:])
            nc.gpsimd.dma_start(pk_sb[0:1, PK_B1:PK_W2], pk[0:1, PK_B1:PK_W2])
            _xf(nc.gpsimd, 0)
            _w1(nc.sync, 0)
            nc.scalar.dma_start(wp1_sb[:], wp1[:])
            _xf(nc.gpsimd, 1)
            _w1(nc.gpsimd, 1)
            _xf(nc.scalar, 2)
            _xf(nc.scalar, 3)
            _w1(nc.sync, 2)
            _w1(nc.sync, 3)
            nc.scalar.dma_start(zrb_sb[:], zrb[:])
            nc.scalar.dma_start(pk_sb[:, PK_W2:], pk[:, PK_W2:])

            # ---- PE warm-up: junk matmuls on memset data -------------------
            psj = ps_sm.tile([1, 512], F32, tag="pp", name="junk")
            for i in range(N_WARM):
                nc.tensor.matmul(psj[0:1, :], jw_sb[0:1, 0:1], jw_sb[0:1, :],
                                 start=True, stop=True)

            # ---- epinet L1 + prior -----------------------------------------
            psz = {}
            psA = {}

            def bz(m):
                for rc in range(RC):
                    t = ps_bz.tile([128, 512], F32, tag="pz",
                                   name=f"pz{m}_{rc}")
                    psz[(m, rc)] = t
                    w8v = z8w_sb[32 * rc:32 * rc + 16, 1024:2048].rearrange(
                        "p (i h) -> p i h", i=2)[:, :, 128 * m:128 * m + 128]
                    z8v = z8w_sb[32 * rc:32 * rc + 16, 0:1024].rearrange(
                        "p (i j) -> p i j", i=2)
                    nc.tensor.matmul(t[:, :], w8v, z8v, start=True, stop=True,
                                     perf_mode=DR, tile_position=(32 * rc, 0))

            def a_gemm(m, ks, stop=False, bias=False):
                if bias:
                    ps = ps_a.tile([128, BL], F32, tag="pa", name=f"pA{m}")
                    psA[m] = ps
                    nc.tensor.matmul(psA[m][:, :],
                                     b1_v[:, 128 * m:128 * m + 128],
                                     ones_sb[0:1, :], start=True, stop=False)
                for i, k in enumerate(ks):
                    nc.tensor.matmul(
                        psA[m][:, :], w13[:, m, k, :], x3[:, k, :],
                        start=False, stop=(stop and i == len(ks) - 1))

            def h_elem(m, act_rcs=()):
                nc.scalar.activation(A_sb[:, BL * m:BL * (m + 1)],
                                     psA[m][:, :], COPY)
                Ab = A_sb[:, BL * m:BL * (m + 1)].unsqueeze(1).broadcast_to(
                    (128, 2, BL))
                us = {}
                for rc in act_rcs:
                    u = tp.tile([128, 512], BF16, tag="u", name=f"u{m}_{rc}")
                    us[rc] = u
                    nc.scalar.activation(u[:], psz[(m, rc)][:, :], COPY)
                ts = []
                for rc in range(RC):
                    t = tp.tile([128, 512], BF16, tag="t")
                    ts.append(t)
                    eng = nc.gpsimd if rc in us else nc.vector
                    srcz = us[rc][:] if rc in us else psz[(m, rc)][:, :]
                    eng.tensor_tensor(
                        t[:].rearrange("p (a b) -> p a b", a=2),
                        srcz.rearrange("p (a b) -> p a b", a=2),
                        Ab, op=ADD)
                for rc in range(RC):
                    eng = nc.vector if rc in us else nc.gpsimd
                    if rc in us:
                        nc.vector.tensor_scalar_max(
                            h_sb[m][:, 512 * rc:512 * rc + 512], ts[rc][:], 0.0)
                    else:
                        nc.gpsimd.tensor_scalar_max(
                            h_sb[m][:, 512 * rc:512 * rc + 512], ts[rc][:], 0.0)

            # m0 first: Bz only needs z8w; A(m0) follows w1[m0]
            bz(0)
            a_gemm(0, [0, 1, 2, 3, 4, 5, 6, 7], bias=True)
            a_gemm(0, [8, 9, 10, 11, 12, 13, 14, 15], stop=True)
            h_elem(0)

            # prior h1 (c0/c1 + wp1 are on-chip by now)
            ps1 = ps_sm.tile([128, 2 * BL], F32, tag="pp", name="pp1")
            for g, (mp, m0) in enumerate([(128, 0), (32, 128)]):
                o = ps1[0:mp, BL * g:BL * g + BL]
                for k in range(8):
                    nc.tensor.matmul(
                        o, wp13[:, k, m0:m0 + mp], x3[:, k, :],
                        start=(k == 0), stop=(k == 7))

            nc.scalar.activation(h1a_sb[:], ps1[0:128, 0:BL], RELU,
                                 bias=zrb_sb[:, ZB_BP1A:ZB_BP1A + 1])
            nc.scalar.activation(h1b_sb[:], ps1[0:32, BL:2 * BL], RELU,
                                 bias=zrb_sb[0:32, ZB_BP1B:ZB_BP1B + 1])

            # m1: Bz between the A halves so m0's psz slots drain first
            a_gemm(1, [0, 1, 2, 3, 4, 5, 6, 7], bias=True)
            bz(1)
            a_gemm(1, [8, 9, 10, 11, 12, 13, 14, 15], stop=True)
            h_elem(1)

            # h2 = relu(h1 @ wp2 + bp2)   (block-diag dense)
            ps2 = ps_sm.tile([128, 2 * BL], F32, tag="pp", name="pp2")
            for g, (mp, m0) in enumerate([(128, 0), (32, 128)]):
                o = ps2[0:mp, BL * g:BL * g + BL]
                nc.tensor.matmul(o, wp2a_v[:, m0:m0 + mp],
                                 h1a_sb[:], start=True, stop=False)
                nc.tensor.matmul(o, wp2b_v[:, m0:m0 + mp],
                                 h1b_sb[:], start=False, stop=True)
            nc.scalar.activation(h2a_sb[:], ps2[0:128, 0:BL], RELU,
                                 bias=zrb_sb[:, ZB_BP2A:ZB_BP2A + 1])
            nc.scalar.activation(h2b_sb[:], ps2[0:32, BL:2 * BL], RELU,
                                 bias=zrb_sb[0:32, ZB_BP2B:ZB_BP2B + 1])

            # m2: rc2/rc3 h-adds go via Act-copy + Pool so DVE keeps up
            a_gemm(2, [0, 1, 2, 3, 4, 5, 6, 7], bias=True)
            bz(2)
            a_gemm(2, [8, 9, 10, 11, 12, 13, 14, 15], stop=True)
            h_elem(2, act_rcs=(3,))

            # prior tail: p + transpose (pS/pT clear the Act queue early)
            psp = ps_p.tile([128, BL], F32, tag="px", name="ppp")
            nc.tensor.matmul(psp[0:32, 0:BL], wp3a_v[:], h2a_sb[:],
                             start=True, stop=False)
            nc.tensor.matmul(psp[0:32, 0:BL], wp3b_v[:], h2b_sb[:],
                             start=False, stop=True)
            nc.scalar.activation(pS_sb[:], psp[0:32, 0:BL], IDENT,
                                 bias=zrb_sb[0:32, ZB_P:ZB_P + 1])
            psT = ps_p.tile([128, 2 * ND], BF16, tag="px", name="psT")
            for half in range(2):
                nc.tensor.transpose(
                    psT[:, ND * half:ND * half + ND],
                    pS_sb[:, 128 * half:128 * half + 128],
                    id_v[0:32, 0:32])
            nc.scalar.activation(pT_sb[:], psT[:, :], COPY)

            bz(3)

            # out2 k0/k1 passes (need only h(m0)/h(m1)).
            # NOTE: start=True zeroes the whole 2KB PSUM region lazily, so
            # only the very first call starts; later chains' first touch
            # inherits the pending-zero (overwrite) semantics.
            pso = ps_sm.tile([128, RT * ND], F32, tag="pp", name="po")

            def o2pass(k):
                for t in range(RT):
                    nc.tensor.matmul(
                        pso[:, ND * t:ND * t + ND],
                        h_sb[k][:, 128 * t:128 * t + 128], w23[:, k, :],
                        start=(k == 0 and t == 0), stop=False,
                        skip_group_check=True)

            # m3: single A chain; psz q1/q2 copied by Act in the Bz window,
            # A3c right at the chain stop, q3 copy after; adds: q0 DVE from
            # PSUM, q1 DVE (bf16 2x), q2/q3 Pool; relus: q0/q1 DVE, q2 Act,
            # q3 Pool.
            a_gemm(3, [0, 1, 2, 3, 4, 5, 6, 7], bias=True)
            u3 = {}
            for rc in (1, 2):
                u = tp.tile([128, 512], BF16, tag="u", name=f"u3_{rc}")
                u3[rc] = u
                nc.scalar.activation(u[:], psz[(3, rc)][:, :], COPY)
            a_gemm(3, [8, 9, 10, 11, 12, 13, 14, 15], stop=True)

            A3c = A_sb[:, 3 * BL:4 * BL]
            nc.scalar.activation(A3c, psA[3][:, :], COPY)
            u = tp.tile([128, 512], BF16, tag="u", name="u3_3")
            u3[3] = u
            nc.scalar.activation(u[:], psz[(3, 3)][:, :], COPY)
            Ab3 = A3c.unsqueeze(1).broadcast_to((128, 2, BL))
            t3s = {}
            for rc in range(RC):
                t3 = tp.tile([128, 512], BF16, tag="t", name=f"t3_{rc}")
                t3s[rc] = t3
                eng = nc.vector if rc < 2 else nc.gpsimd
                srcz = (psz[(3, 0)][:, :] if rc == 0 else u3[rc][:])
                eng.tensor_tensor(
                    t3[:].rearrange("p (a b) -> p a b", a=2),
                    srcz.rearrange("p (a b) -> p a b", a=2), Ab3, op=ADD)
            relu_eng = [nc.vector, nc.vector, nc.scalar, nc.gpsimd]
            for rc in range(RC):
                dst = h_sb[3][:, 512 * rc:512 * rc + 512]
                if relu_eng[rc] is nc.scalar:
                    nc.scalar.activation(dst, t3s[rc][:], RELU)
                else:
                    relu_eng[rc].tensor_scalar_max(dst, t3s[rc][:], 0.0)

            o2pass(0)
            o2pass(1)
            o2pass(2)

            # ---- per-chunk tail: k3+Id, then Pool gm-mult, DVE row-sum ------
            for q in range(4):
                for t in range(4 * q, 4 * q + 4):
                    o = pso[:, ND * t:ND * t + ND]
                    nc.tensor.matmul(
                        o, h_sb[3][:, 128 * t:128 * t + 128],
                        w23[:, 3, :], start=False, stop=False,
                        skip_group_check=True)
                    nc.tensor.matmul(
                        o, id_v[:], pT_sb[:, ND * (t % 2):ND * (t % 2) + ND],
                        start=False, stop=True, skip_group_check=True)
                sl = slice(4 * ND * q, 4 * ND * (q + 1))
                nc.vector.tensor_tensor(gm_sb[:, sl], pso[:, sl],
                                        zrb_sb[:, sl], op=MULT)
                nc.vector.tensor_reduce(
                    res_sb[:, 4 * q:4 * q + 4],
                    gm_sb[:, sl].rearrange("p (t o) -> p t o", o=ND),
                    axis=mybir.AxisListType.X, op=ADD)
            nc.sync.dma_start(out[:], res_sb[:])

            if DEBUG_TAPS:
                d_h0 = nc.dram_tensor("d_h0", [128, R], BF16,
                                      kind="ExternalOutput").ap()
                d_pS = nc.dram_tensor("d_pS", [32, BL], BF16,
                                      kind="ExternalOutput").ap()
                d_gm = nc.dram_tensor("d_gm", [128, RT * ND], BF16,
                                      kind="ExternalOutput").ap()
                nc.sync.dma_start(d_h0[:], h_sb[0][:])
                nc.sync.dma_start(d_pS[:], pS_sb[:])
                nc.sync.dma_start(d_gm[:], gm_sb[:])

    nc.compile()
    return nc


def _prep(x, feature, z, Wep1, bep1, Wep2, bep2, Wp1, bp1, Wp2, bp2, Wp3, bp3):
    """Host-side weight/layout prep shared across cores."""
    xfT = np.ascontiguousarray(
        np.concatenate([x, feature], axis=1).T.astype(NPDT))  # [XF, B]
    # swizzle w1 into SBUF layout [p, m, (k h)]
    w1 = np.ascontiguousarray(
        np.asarray(Wep1, np.float32)[:XF].astype(NPDT)
        .reshape(KT, 128, MT, 128).transpose(1, 2, 0, 3)
        .reshape(128, MT, KT * 128))

    pk = np.zeros((128, PK_COLS), NPDT)
    pk[0, PK_B1:PK_W2] = np.asarray(bep1, np.float32)
    pk[:, PK_W2:PK_ID] = (np.asarray(Wep2, np.float32)
                          .reshape(4, 128, ND).transpose(1, 0, 2)
                          .reshape(128, 4 * ND))
    pk[:, PK_ID:PK_W2A] = np.eye(128, dtype=np.float32)
    wp2 = np.zeros((PHF, PHF), np.float32)
    wp3 = np.zeros((PHF, ND), np.float32)
    for e in range(ND):
        wp2[5 * e:5 * e + 5, 5 * e:5 * e + 5] = Wp2[e]
        wp3[5 * e:5 * e + 5, e] = np.asarray(Wp3)[e, :, 0]
    pk[:, PK_W2A:PK_W2B] = wp2[0:128]
    pk[0:32, PK_W2B:PK_W3A] = wp2[128:160]
    pk[:, PK_W3A:PK_W3B] = wp3[0:128]
    pk[0:32, PK_W3B:PK_COLS] = wp3[128:160]

    wp1 = np.ascontiguousarray(
        np.asarray(Wp1, np.float32).transpose(1, 0, 2).reshape(SD, PHF)
        .reshape(8, 128, PHF).transpose(1, 0, 2)
        .reshape(128, 8 * PHF).astype(NPDT))

    w1z8 = (np.asarray(Wep1, np.float32)[XF:]
            .reshape(16, 2, EH).reshape(16, 2 * EH).astype(NP8))

    bp1f = np.asarray(bp1, np.float32).reshape(PHF)
    bp2f = np.asarray(bp2, np.float32).reshape(PHF)
    pbias = np.asarray(bep2, np.float32) + np.asarray(bp3, np.float32)[:, 0]

    shared = dict(w1=w1, pk=pk, wp1=wp1)
    in_maps = []
    for c in range(N_CORES):
        sl = slice(c * BL, (c + 1) * BL)
        zTf = np.asarray(z)[sl].transpose(1, 0, 2).reshape(R, ND).T  # [32, R]
        z8w = np.zeros((128, 2048), NP8)
        for rc in range(RC):
            z8w[32 * rc:32 * rc + 16, 0:1024] = (
                zTf[:, 512 * rc:512 * rc + 512]
                .reshape(16, 2, 512).reshape(16, 1024).astype(NP8))
            z8w[32 * rc:32 * rc + 16, 1024:2048] = w1z8
        zrb = np.zeros((128, ZB_COLS), np.float32)
        # zrb[p, (t, o)] = z[o, 128 t + p]
        zrb[:, 0:RT * ND] = (zTf.T.reshape(RT, 128, ND)
                             .transpose(1, 0, 2).reshape(128, RT * ND))
        zrb[:, ZB_BP1A] = bp1f[:128]
        zrb[0:32, ZB_BP1B] = bp1f[128:]
        zrb[:, ZB_BP2A] = bp2f[:128]
        zrb[0:32, ZB_BP2B] = bp2f[128:]
        zrb[0:32, ZB_P] = pbias
        m = dict(shared)
        m["xfT"] = np.ascontiguousarray(
            xfT[:, sl].reshape(KT, 128, BL).transpose(1, 0, 2)
            .reshape(128, KT * BL))
        m["z8w"] = z8w
        m["zrb"] = zrb
        in_maps.append(m)
    return in_maps


def kernel(**inputs):
    if "nc" not in _CACHE:
        _CACHE["nc"] = _build()
    nc = _CACHE["nc"]
    in_maps = _prep(**inputs)
    last_err = None
    for _attempt in range(3):
        try:
            res = run_bass_kernel_spmd(nc, in_maps, list(range(N_CORES)))
            full = np.empty((B, N_Z, 1), np.float32)
            for c in range(N_CORES):
                S = np.asarray(res.results[c]["out"])  # [128, RT]
                r = S.T.reshape(R)                     # r = 128 t + p
                full[c * BL:(c + 1) * BL, :, 0] = r.reshape(N_Z, BL).T
            return full
        except Exception as e:  # transient device/transfer hiccups
            last_err = e
            time.sleep(5.0 * (_attempt + 1))
    raise last_err
